# revision 34
# baseline (speedup 1.0000x reference)
"""Trainium2 Bass kernel for LLN+diag attention.

out = 0.5 * (lln_linear_attention(q,k,v) + block_diag_attention(q,k,v))

Shapes: q,k,v [4,16,4096,64] fp32.  8 NeuronCores, one (B*H)/8 = 8-head
shard per core (batch+head parallel); both paths are independent per
head so there is no cross-device communication.

Host prep (sharding/layout/dtype only): the two global scalars
sigma_q/sigma_k (std over the whole tensor, inherently cross-device) are
folded into the shipped operands.  Operands are pre-permuted on host and
merged so every device DMA is a large contiguous 2D copy:
  qk  [pair, 128, 2, N]        bf16  [:,:,0]=(alpha*q)^T  [:,:,1]=(k/(8*alpha))^T
                                     partition = hh*64+d
  kvb [pair, 128, 2, NT, 2D+1] bf16  [...,0:64]=beta*k  [...,64:128]=v
                                     [...,128]=2.0 aug col, partition = n%128
  out [pair, 128, NT, 2, D]    bf16  nt-major so group-adds and half-pair
                                     DMAs touch disjoint ranges

Math identities: row-max / global-max subtraction before exp cancels in
both paths' ratios (exponents <= ~12.5, fp32 safe); EPS=1e-8 is ~1e-9
relative and dropped; the V-augmentation column of 2.0 doubles both
denominators so adding the two divided halves gives 0.5*(lin+diag).

Schedule (measured ~98-100us on HW, vs 103us for the per-pair-serial
version this evolved from):
  - per head pair, 8 groups of 512 positions; each group: block-diag
    scores (PE) -> at-exp (Act) -> AV+QKV aug matmuls (PE) -> two
    recip+mult divides (DVE) -> combine add (Pool) -> streamed out-DMA.
  - scores/at-exp are batched TWO groups at a time into per-head 1-bank
    64-packed PSUM tiles: one at-exp per head per 2 groups halves Act
    instr overhead, and the separate per-head tiles stagger the
    at-exp->scores WAR cycles of the two heads (the per-group critical
    cycle is at-exp -> next scores -> next at-exp under sc bufs=1).
  - PSUM (8 banks): sc0 1 + sc1 1 + da 2 + li 2 + kv 1.  Bank-feeding
    rule respected everywhere: streams from different PE row-groups
    never share a bank unless their output partitions are disjoint.
  - pair p+1's whole KV phase (ke exps in 1024-col chunks, 64 KV
    matmuls, kva PSUM->SBUF copy) plus all qte exp chunks are emitted as
    "fillers" interleaved into pair p's group loop, at most ~1.15us of
    Act filler per group, EMITTED BEFORE the group body so the Act FIFO
    head is never a waiting at-exp with ready work stuck behind it.
  - inputs are prefetched 2 pairs ahead (bufs=3 pools, 4 DMA triggers
    per pair); output accumulates in an nt-major SBUF tile and streams
    out per 2 groups.  Engines: PE matmuls; Act exps only; DVE divides
    + kva copy; Pool combine-adds.
"""

import math
import os
import sys

for _p in ("/opt/trn_rl_repo", "/opt/pypackages"):
    if os.path.isdir(_p) and _p not in sys.path:
        sys.path.insert(0, _p)

import numpy as np
import ml_dtypes

B, H, N, D = 4, 16, 4096, 64
N_CORES = 8
HPC = (B * H) // N_CORES          # heads per core = 8
P2 = HPC // 2                     # head pairs per core = 4
NT = N // 128                     # 128-row n-tiles per head = 32
GROUPS = 8                        # groups per head
GNT = NT // GROUPS                # n-tiles per group = 4
NG = N // GROUPS                  # columns per group = 512
A_CONST = 0.14855178144710912
B_CONST = -0.35487039130661086

_BF16 = ml_dtypes.bfloat16

_cache = {}


def _build():
    import concourse.bass as bass
    import concourse.bacc as bacc
    import concourse.mybir as mybir
    from concourse.tile import TileContext

    dt = mybir.dt
    F32, BF = dt.float32, dt.bfloat16
    Exp = mybir.ActivationFunctionType.Exp
    Copy = mybir.ActivationFunctionType.Copy
    MUL = mybir.AluOpType.mult
    ADD = mybir.AluOpType.add

    nc = bacc.Bacc()
    qk_d = nc.dram_tensor("qk", [P2, 128, 2, N], BF, kind="ExternalInput")
    kvb_d = nc.dram_tensor("kvb", [P2, 128, 2, NT, 2 * D + 1], BF, kind="ExternalInput")
    out_d = nc.dram_tensor("out", [P2, 128, NT, 2, D], BF, kind="ExternalOutput")

    with TileContext(nc) as tc:
        from contextlib import ExitStack

        with ExitStack() as ctx:
            qk_p = ctx.enter_context(tc.tile_pool(name="qkp", bufs=3))
            kvb_p = ctx.enter_context(tc.tile_pool(name="kvbp", bufs=3))
            qte_p = ctx.enter_context(tc.tile_pool(name="qtep", bufs=2))
            ke_p = ctx.enter_context(tc.tile_pool(name="kep", bufs=2))
            sm_p = ctx.enter_context(tc.tile_pool(name="small", bufs=2))
            at_p = ctx.enter_context(tc.tile_pool(name="attn", bufs=4))
            t_p = ctx.enter_context(tc.tile_pool(name="tmp", bufs=6))
            r_p = ctx.enter_context(tc.tile_pool(name="recip", bufs=8))
            o_p = ctx.enter_context(tc.tile_pool(name="outp", bufs=2))
            sc_ps_p = ctx.enter_context(tc.tile_pool(name="scps", bufs=1, space="PSUM"))
            da_ps_p = ctx.enter_context(tc.tile_pool(name="daps", bufs=1, space="PSUM"))
            li_ps_p = ctx.enter_context(tc.tile_pool(name="lips", bufs=1, space="PSUM"))
            kv_ps_p = ctx.enter_context(tc.tile_pool(name="kvps", bufs=1, space="PSUM"))

            C0 = 2 * NG  # first qk chunk: covers groups 0-1

            qks = [None] * P2
            kvbs = [None] * P2
            qtes = [None] * P2
            kvas = [None] * P2
            ohs = [None] * P2
            at2s = [None] * P2

            def emit_inputs(p):
                kvb = kvb_p.tile([128, 2, NT, 2 * D + 1], BF, tag="kvb", name="kvb")
                nc.sync.dma_start(kvb[:, 0], kvb_d[p][:, 0])
                qk = qk_p.tile([128, 2, N], BF, tag="qk", name="qk")
                nc.sync.dma_start(qk[:, :, 0:C0], qk_d[p][:, :, 0:C0])
                nc.sync.dma_start(kvb[:, 1], kvb_d[p][:, 1])
                nc.sync.dma_start(qk[:, :, C0:N], qk_d[p][:, :, C0:N])
                qks[p], kvbs[p] = qk, kvb

            def emit_ke(p, chunk):
                # 1024-col chunks, chunk=(hh<<1)|half: each fits in the
                # ~1.4us/group Act slack so at-exps are never starved
                if kes[p] is None:
                    kes[p] = ke_p.tile([128, 2, NT, D], BF, tag="ke", name="ke")
                hh, half = chunk >> 1, chunk & 1
                a0, a1 = half * (NT // 2), (half + 1) * (NT // 2)
                nc.scalar.activation(
                    kes[p][:, hh, a0:a1], kvbs[p][:, hh, a0:a1, 0:D], Exp
                )

            kes = [None] * P2

            def emit_kv(p, a0, a1):
                # KV_aug[d, e|S] accumulation over n-tiles a0:a1, both
                # heads interleaved for LDW overlap.
                if a0 == 0:
                    kv_ps = kv_ps_p.tile(
                        [128, D + 1], F32, tag="kv", name="kv", padded_shape=[128, 512]
                    )
                    kv_tiles[p] = kv_ps
                kv_ps = kv_tiles[p]
                for a in range(a0, a1):
                    for hh in range(2):
                        nc.tensor.matmul(
                            kv_ps[64 * hh : 64 * hh + 64, :],
                            lhsT=kes[p][:, hh, a, :],
                            rhs=kvbs[p][:, hh, a, D : 2 * D + 1],
                            start=(a == 0),
                            stop=(a == NT - 1),
                            tile_position=(0, 64 * hh),
                            skip_group_check=True,
                        )

            def emit_kva(p):
                kva = sm_p.tile([128, D + 1], BF, tag="kva", name="kva")
                nc.vector.tensor_copy(kva[:], kv_tiles[p][:])
                kvas[p] = kva

            kv_tiles = [None] * P2

            def emit_qte(p, c):
                # exp of qt columns [c, c+1024)
                if c == 0:
                    qtes[p] = qte_p.tile([128, N], BF, tag="qte", name="qte")
                nc.scalar.activation(
                    qtes[p][:, c : c + 1024],
                    qks[p][:, 0, c : c + 1024],
                    Exp,
                )

            def emit_scores2(p, G):
                # scores^T + exp for groups 2G and 2G+1: per-head 1-bank
                # sc tiles (64-packed, 8 slots = exactly one bank; the two
                # feeding positions (hp,0)/(hp,64) share a row-group and
                # write disjoint partitions, same pattern as the kv bank).
                # One at-exp per head per TWO groups halves the Act
                # instr-overhead of the exps.
                qk = qks[p]
                at2 = []
                for hh in range(2):
                    hp = 64 * hh
                    sc = sc_ps_p.tile(
                        [128, 2 * GNT, 64], F32, tag=f"sc{hh}", name="sc",
                        padded_shape=[128, 2 * GNT, 64],
                    )
                    for j in range(4 * GNT):
                        a = 2 * GNT * G + (j >> 1)
                        half = j & 1
                        b = 2 * a + half
                        nc.tensor.matmul(
                            sc[64 * half : 64 * half + 64, j >> 1, :],
                            lhsT=qk[hp : hp + 64, 1, 64 * b : 64 * b + 64],
                            rhs=qk[hp : hp + 64, 0, 64 * b : 64 * b + 64],
                            start=True,
                            stop=True,
                            tile_position=(hp, 64 * half),
                        )
                    at_sb = at_p.tile([128, 2 * GNT, 64], BF, tag=f"at{hh}", name="at")
                    nc.scalar.activation(at_sb[:], sc[:], Exp)
                    at2.append(at_sb)
                return at2

            def emit_diag(p, g, at2):
                kvb = kvbs[p]
                sub = g & 1
                at_list = [a[:, sub * GNT : (sub + 1) * GNT, :] for a in at2]

                # -- diag out_aug + divide --
                da_f = da_ps_p.tile([128, 1024], F32, tag="da", name="da")
                dav = (
                    da_f.rearrange("p (h y) -> p h y", h=2)[:, :, 0:340]
                    .rearrange("p h (s x) -> p h s x", x=85)
                )
                for i in range(GNT):
                    for hh in range(2):
                        for half in range(2):
                            nc.tensor.matmul(
                                dav[64 * half : 64 * half + 64, hh, i, 0 : D + 1],
                                lhsT=at_list[hh][64 * half : 64 * half + 64, i, :],
                                rhs=kvb[
                                    64 * half : 64 * half + 64,
                                    hh,
                                    GNT * g + i,
                                    D : 2 * D + 1,
                                ],
                                start=True,
                                stop=True,
                                tile_position=(64 * half, 64 * half),
                            )
                rd = r_p.tile([128, 2, GNT], F32, tag="rd", name="rd")
                nc.vector.reciprocal(rd[:], dav[:, :, :, D])
                t2 = t_p.tile([128, 2, GNT, D], BF, tag="t2", name="t2")
                nc.vector.tensor_tensor(
                    t2[:], dav[:, :, :, 0:D],
                    rd[:].to_broadcast((128, 2, GNT, D)), op=MUL,
                )
                return t2

            def emit_lin(p, g, t2):
                # -- linear path out_aug + divide --
                li_f = li_ps_p.tile([128, 1024], F32, tag="li", name="li")
                liv = (
                    li_f.rearrange("p (h y) -> p h y", h=2)[:, :, 0:340]
                    .rearrange("p h (s x) -> p h s x", x=85)
                )
                for i in range(GNT):
                    a = GNT * g + i
                    for hh in range(2):
                        hp = 64 * hh
                        nc.tensor.matmul(
                            liv[:, hh, i, 0 : D + 1],
                            lhsT=qtes[p][hp : hp + 64, 128 * a : 128 * a + 128],
                            rhs=kvas[p][hp : hp + 64, :],
                            start=True,
                            stop=True,
                            tile_position=(hp, 0),
                        )
                rl = r_p.tile([128, 2, GNT], F32, tag="rl", name="rl")
                nc.vector.reciprocal(rl[:], liv[:, :, :, D])
                t1 = t_p.tile([128, 2, GNT, D], BF, tag="t1", name="t1")
                nc.vector.tensor_tensor(
                    t1[:], liv[:, :, :, 0:D],
                    rl[:].to_broadcast((128, 2, GNT, D)), op=MUL,
                )

                # -- combine into the pair's output tile; DMA out per
                # half-pair (4KB contiguous per partition — per-group
                # 512B packets measured only ~66 GB/s).  nt-major so each
                # group add and each half-pair DMA touch disjoint ranges.
                if g == 0:
                    ohs[p] = o_p.tile([128, NT, 2, D], BF, tag="o", name="o")
                oslice = ohs[p][:, GNT * g : GNT * (g + 1), :, :].rearrange(
                    "p s h x -> p h s x"
                )
                nc.gpsimd.tensor_tensor(oslice, t1[:], t2[:], op=ADD)
                # out triggers alternate issuing engines so the transfers
                # land on different DGE queues (a single queue moves only
                # ~55 GB/s; 1MB/pair on one queue was at the pair period)
                if p == P2 - 1 and g >= GROUPS - 2:
                    lo = g * GNT
                    for hp, eng in ((0, nc.gpsimd), (64, nc.sync)):
                        eng.dma_start(
                            out_d[p][hp : hp + 64, lo : lo + GNT, :, :],
                            ohs[p][hp : hp + 64, lo : lo + GNT, :, :],
                        )
                elif g % 2 == 1:
                    lo = (g - 1) * GNT
                    eng = nc.gpsimd if (g // 2) % 2 == 0 else nc.sync
                    eng.dma_start(
                        out_d[p][:, lo : lo + 2 * GNT, :, :],
                        ohs[p][:, lo : lo + 2 * GNT, :, :],
                    )

            # ---- prologue: pair 0/1 inputs; pair 0's groups 0-1 diag
            # BEFORE its KV phase so the early DVE divides are not
            # head-of-line blocked, and the PE has work during ke exps ----
            emit_inputs(0)
            if P2 > 1:
                emit_inputs(1)
            emit_ke(0, 0)
            emit_ke(0, 2)
            at2_0 = emit_scores2(0, 0)
            t2_0 = emit_diag(0, 0, at2_0)
            emit_kv(0, 0, NT // 2)
            emit_ke(0, 1)
            emit_ke(0, 3)
            t2_1 = emit_diag(0, 1, at2_0)
            emit_kv(0, NT // 2, NT)
            emit_qte(0, 0)
            emit_kva(0)

            # ---- pair loop: pair p's groups, with pair p+1's KV phase
            # and pair p+2's input DMA interleaved ----
            for p in range(P2):
                for g in range(GROUPS):
                    # fillers FIRST each group: their deps are satisfied
                    # long ago, so the Act FIFO head is never a waiting
                    # at-exp with ready work stuck behind it
                    if g == 0 and p + 2 < P2:
                        emit_inputs(p + 2)
                    # one 1024-col Act filler chunk per group; KV
                    # matmuls in quarter chunks so no group's PE load
                    # doubles (the kv groups were the period spikes)
                    if g == 0:
                        emit_qte(p, 1024)
                    elif g == 2:
                        emit_qte(p, 2048)
                    elif g == 4:
                        emit_qte(p, 3072)
                    if p + 1 < P2:
                        if g == 1:
                            emit_ke(p + 1, 0)
                        elif g == 3:
                            emit_ke(p + 1, 2)
                            emit_kv(p + 1, 0, NT // 4)
                        elif g == 4:
                            emit_kv(p + 1, NT // 4, NT // 2)
                        elif g == 5:
                            emit_ke(p + 1, 1)
                        elif g == 6:
                            emit_ke(p + 1, 3)
                            emit_kv(p + 1, NT // 2, 3 * NT // 4)
                        elif g == 7:
                            emit_qte(p + 1, 0)
                            emit_kv(p + 1, 3 * NT // 4, NT)
                            emit_kva(p + 1)
                    if p == 0 and g == 0:
                        emit_lin(0, 0, t2_0)
                    elif p == 0 and g == 1:
                        emit_lin(0, 1, t2_1)
                    else:
                        if g % 2 == 0:
                            at2s[p] = emit_scores2(p, g // 2)
                        emit_lin(p, g, emit_diag(p, g, at2s[p]))

    nc.finalize()
    return nc


def _get_nc():
    if "nc" not in _cache:
        _cache["nc"] = _build()
    return _cache["nc"]


def _prep(q, k, v):
    q = np.asarray(q, dtype=np.float32)
    k = np.asarray(k, dtype=np.float32)
    v = np.asarray(v, dtype=np.float32)
    sq = float(np.std(q.astype(np.float64), ddof=1))
    sk = float(np.std(k.astype(np.float64), ddof=1))
    st = math.sqrt((sq * sq * sk * sk - B_CONST) / (2.0 * A_CONST))
    alpha = st / sq
    beta = st / sk

    BH = B * H
    qf = q.reshape(BH, N, D)
    kf = k.reshape(BH, N, D)
    vf = v.reshape(BH, N, D)
    # qk: [BH//2, 128, 2, N]
    qt = (alpha * qf).astype(_BF16).transpose(0, 2, 1).reshape(BH // 2, 128, N)
    kt = (
        (kf * (1.0 / (8.0 * alpha)))
        .astype(_BF16)
        .transpose(0, 2, 1)
        .reshape(BH // 2, 128, N)
    )
    qk = np.empty((BH // 2, 128, 2, N), dtype=_BF16)
    qk[:, :, 0, :] = qt
    qk[:, :, 1, :] = kt
    # kvb: [BH//2, 128, 2, NT, 2D+1]
    kb = (beta * kf).astype(_BF16).reshape(BH, NT, 128, D).transpose(0, 2, 1, 3)
    vb = vf.astype(_BF16).reshape(BH, NT, 128, D).transpose(0, 2, 1, 3)
    kvb = np.empty((BH // 2, 128, 2, NT, 2 * D + 1), dtype=_BF16)
    kvb[:, :, 0, :, 0:D] = kb[0::2]
    kvb[:, :, 1, :, 0:D] = kb[1::2]
    kvb[:, :, 0, :, D : 2 * D] = vb[0::2]
    kvb[:, :, 1, :, D : 2 * D] = vb[1::2]
    kvb[:, :, :, :, 2 * D] = np.float32(2.0)

    in_maps = []
    for c in range(N_CORES):
        ps = slice(c * P2, (c + 1) * P2)
        in_maps.append(
            {
                "qk": np.ascontiguousarray(qk[ps]),
                "kvb": np.ascontiguousarray(kvb[ps]),
            }
        )
    return in_maps


def run_on_device(in_maps, **kw):
    from concourse.bass_utils import run_bass_kernel_spmd

    return run_bass_kernel_spmd(_get_nc(), in_maps, core_ids=list(range(N_CORES)), **kw)


def kernel(q, k, v):
    in_maps = _prep(q, k, v)
    res = run_on_device(in_maps)
    # res[c]["out"]: [P2, 128, 2, NT, D] -> heads [P2,2] n=(nt*128+part)
    outs = []
    for r in res.results:
        o = r["out"]  # [P2, 128, NT, 2, D]
        o = o.transpose(0, 3, 2, 1, 4).reshape(HPC, N, D)
        outs.append(o)
    out = np.concatenate(outs, axis=0)
    return np.ascontiguousarray(out.reshape(B, H, N, D)).astype(np.float32)


if __name__ == "__main__":
    nc = _get_nc()
    print("built ok")
